# revision 32
# baseline (speedup 1.0000x reference)
"""LoraLinear (x @ W.T + 2*(x @ A.T) @ B.T) on 8 TRN2 NeuronCores.

Tensor-parallel: W and lora_B sharded row-wise (out_features) across the
8 cores; x replicated. The rank-64 intermediate u = 2*(x @ A.T) is
identical on every core, so it is computed host-side (16 MFLOP) and
uploaded as 16 KiB instead of replicating the 1 MiB lora_A — the kernel
is DMA-bound, so replicated bytes are the scarce resource. Each core
streams its W.T shard with contiguous 1 MiB DMAs (32 MiB/core, the
memory-bound term) while x.T tiles sit stationary in the PE.

The whole W/x/u/B operand path is declared float32r (same fp32 bits
host-side): the PE fast path runs 1 cycle/row at moving>=512 vs 4 for
plain fp32, which moves the kernel from PE-bound (~135 us) to DMA-bound
(~85 us). The walrus BIR verifier requires f32r matmul operands to be
produced in f32r dtype end to end, hence the dtype on the DRAM/SBUF
tensors rather than a bitcast at the matmul site.

Raw Bass (no Tile); synchronization is explicit standalone wait_ge
instructions on a handful of semaphores.

Self-contained: shapes hardcoded for
  x [64, 4096] f32, weight [16384, 4096] f32,
  lora_A [64, 4096] f32, lora_B [16384, 64] f32  ->  out [64, 16384] f32
"""

import numpy as np

import concourse.bass as bass
import concourse.mybir as mybir
from concourse.bass_utils import run_bass_kernel_spmd

N_CORES = 8
TOK = 64          # tokens
IN_F = 4096       # in_features (contraction)
OUT_F = 16384     # out_features
R = 64            # lora rank
SCALING = 2.0
O_SHARD = OUT_F // N_CORES   # 2048 out features per core
P = 128
KT = IN_F // P               # 32 k-tiles
NB = O_SHARD // 512          # 4 psum blocks of 512
NBUF = 8                     # W slab prefetch ring depth
GATE = 1                     # gate ring reuse every GATE slabs (1 wait/GATE)
F32 = mybir.dt.float32
USE_F32R = True
INPUTS_ON_ACT = True    # xt/ut/bt DMAs on the scalar (Activation) queue
W_HALVES = 1            # full-slab W DMAs (half-slabs lose ~1us in situ)


def _build_nc(reps=1, w_only=False):
    """reps>1 loops the whole computation inside one NEFF (bench only):
    per-iteration semaphore targets are offset so the pipeline stays
    correct across iterations. w_only=True loads xt/ut/bt once and
    re-streams only W per iteration (bandwidth calibration)."""
    nc = bass.Bass()
    # Host-prepared layouts (see _prep_in_maps):
    #   xt  [128, KT*64]  x.T in SBUF partition-major k-tile layout
    #   ut  [64, 64]      (SCALING * x @ lora_A.T).T  (r rows, t cols)
    #   wt  [4096, 2048]  per-core W shard, transposed (k rows, o cols)
    #   bt  [64, 2048]    per-core lora_B shard, transposed (r rows, o cols)
    FIN = mybir.dt.float32r if USE_F32R else F32
    xt = nc.dram_tensor("xt", [P, KT * TOK], FIN, kind="ExternalInput")
    ut = nc.dram_tensor("ut", [R, TOK], FIN, kind="ExternalInput")
    wt = nc.dram_tensor("wt", [IN_F, O_SHARD], FIN, kind="ExternalInput")
    bt = nc.dram_tensor("bt", [R, O_SHARD], FIN, kind="ExternalInput")
    out = nc.dram_tensor("out", [TOK, O_SHARD], F32, kind="ExternalOutput")

    # DMA completion semaphores are per-source: walrus may spread dma_starts
    # over both HW-DGE rings, so completions of different DMAs can reorder
    # and a shared counter would fire early. Each W ring slot gets its own
    # semaphore; xt/ut/bt each get their own.
    import contextlib
    with contextlib.ExitStack() as stack:
        ec = stack.enter_context
        xt_sb = ec(nc.sbuf_tensor("xt_sb", [P, KT, TOK], FIN))
        ut_sb = ec(nc.sbuf_tensor("ut_sb", [R, TOK], FIN))
        bt_sb = ec(nc.sbuf_tensor("bt_sb", [R, O_SHARD], FIN))
        w_sb = ec(nc.sbuf_tensor("w_sb", [P, NBUF, O_SHARD], FIN))
        out_sb = ec(nc.sbuf_tensor("out_sb", [TOK, O_SHARD], F32))
        ps_o = ec(nc.psum_tensor("ps_o", [TOK, NB, 512], F32))
        xt_sem = ec(nc.semaphore("xt_sem"))    # xt DMA done (+16)
        ut_sem = ec(nc.semaphore("ut_sem"))    # ut DMA done (+16)
        bt_sem = ec(nc.semaphore("bt_sem"))    # bt DMA done (+16)
        w_sems = [ec(nc.semaphore(f"w_sem{s}"))  # slab DMA done, per slot
                  for s in range(NBUF)]
        slot_sem = ec(nc.semaphore("slot_sem"))  # PE done with slab k (+1)
        pe_sem = ec(nc.semaphore("pe_sem"))    # bank stop-matmuls (+1)
        cp_sem = ec(nc.semaphore("cp_sem"))    # DVE copybacks done (+1)
        done_sem = ec(nc.semaphore("done_sem"))  # out DMA done (+16)
        block = ec(nc.Block())

        @block.sync
        def _(sync):
            # W slab stream, all reps back-to-back. Output DMAs live on the
            # gpsimd queue (and inputs optionally on the scalar queue) so
            # the W stream never stalls on compute completion.
            for it in range(reps):
                if not INPUTS_ON_ACT and (it == 0 or not w_only):
                    if it > 0:
                        sync.wait_ge(slot_sem, it * KT)
                    sync.dma_start(
                        out=xt_sb[:], in_=xt.rearrange("p (kt t) -> p kt t", kt=KT)
                    ).then_inc(xt_sem, 16)
                    sync.dma_start(out=ut_sb[:], in_=ut[:]).then_inc(ut_sem, 16)
                    sync.dma_start(out=bt_sb[:], in_=bt[:]).then_inc(bt_sem, 16)
                for k in range(KT):
                    gk = it * KT + k
                    if gk >= NBUF and gk % GATE == 0:
                        # covers slots for slabs gk..gk+GATE-1
                        sync.wait_ge(slot_sem, gk - NBUF + GATE)
                    # half-slab W DMAs: 4 KiB descriptors measured ~1 us/
                    # iter faster than one 8 KiB-descriptor slab DMA; each
                    # incs the slot sem, the PE waits on the total
                    hw = O_SHARD // W_HALVES
                    for h in range(W_HALVES):
                        sync.dma_start(
                            out=w_sb[:, gk % NBUF, h * hw:(h + 1) * hw],
                            in_=wt[k * P:(k + 1) * P, h * hw:(h + 1) * hw],
                        ).then_inc(w_sems[gk % NBUF], 16)
            sync.wait_ge(done_sem, 16 * NB * reps)

        def issue_stores(q, it):
            # Per-bank output store: bank b leaves as soon as its copyback
            # lands, overlapping the tail of the W stream.
            base_cp = it * NB
            for b in range(NB):
                q.wait_ge(cp_sem, base_cp + 1 + b)
                q.dma_start(
                    out=out[:, b * 512:(b + 1) * 512],
                    in_=out_sb[:, b * 512:(b + 1) * 512],
                ).then_inc(done_sem, 16)

        if INPUTS_ON_ACT:
            # Inputs AND output stores on the Activation queue: both ride
            # the qActDynamicHW hardware-DGE ring. gpsimd dma_start goes
            # through SWDGE (Q7 software descriptor generation), which puts
            # DSP latency on the critical tail — avoid it for the stores.
            @block.scalar
            def _(scalar):
                for it in range(reps):
                    if it == 0 or not w_only:
                        if it > 0:
                            scalar.wait_ge(slot_sem, it * KT)
                        scalar.dma_start(
                            out=xt_sb[:],
                            in_=xt.rearrange("p (kt t) -> p kt t", kt=KT),
                        ).then_inc(xt_sem, 16)
                        scalar.dma_start(out=ut_sb[:], in_=ut[:]).then_inc(ut_sem, 16)
                        scalar.dma_start(out=bt_sb[:], in_=bt[:]).then_inc(bt_sem, 16)
                    issue_stores(scalar, it)
        else:
            @block.gpsimd
            def _(gpsimd):
                for it in range(reps):
                    issue_stores(gpsimd, it)

        @block.tensor
        def _(tensor):
            # Prologue: psum[t, o] = uT.T @ bT with start=True, so the
            # k-loop's last matmul carries stop and the tail is minimal.
            # pe_sem: +1 per stop-matmul of banks 0..NB-2; bank NB-1's stop
            # doubles as the slab-31 slot release on slot_sem (which the
            # last copyback keys on).
            for it in range(reps):
                base_1 = 0 if w_only else it * 16
                tensor.wait_ge(xt_sem, base_1 + 16)    # xt resident
                tensor.wait_ge(ut_sem, base_1 + 16)    # ut resident
                tensor.wait_ge(bt_sem, base_1 + 16)    # bt resident
                for b in range(NB):
                    if it > 0:
                        # prior iteration's bank-b copyback must finish
                        tensor.wait_ge(cp_sem, (it - 1) * NB + 1 + b)
                    nc.tensor.matmul(
                        ps_o[:, b, :], ut_sb[:],
                        bt_sb[:, b * 512:(b + 1) * 512],
                        start=True, stop=False)
                for k in range(KT):
                    gk = it * KT + k
                    tensor.wait_ge(w_sems[gk % NBUF], 16 * W_HALVES * (gk // NBUF + 1))
                    for b in range(NB):
                        mm = nc.tensor.matmul(
                            ps_o[:, b, :], xt_sb[:, k, :],
                            w_sb[:, gk % NBUF, b * 512:(b + 1) * 512],
                            start=False, stop=(k == KT - 1))
                        if k == KT - 1 and b < NB - 1:
                            mm.then_inc(pe_sem, 1)
                        elif b == NB - 1:
                            mm.then_inc(slot_sem, 1)

        @block.vector
        def _(vector):
            for it in range(reps):
                base_pe = it * (NB - 1)
                for b in range(NB):
                    if b < NB - 1:
                        vector.wait_ge(pe_sem, base_pe + 1 + b)  # bank stopped
                    else:
                        vector.wait_ge(slot_sem, (it + 1) * KT)  # slab31 done
                    if it > 0 and b == 0:
                        # all prior-iteration stores must have left SBUF
                        # (total count: order-safe under DMA reordering)
                        vector.wait_ge(done_sem, it * 16 * NB)
                    nc.vector.tensor_copy(
                        out=out_sb[:, b * 512:(b + 1) * 512], in_=ps_o[:, b, :]
                    ).then_inc(cp_sem, 1)

    return nc


_NC_CACHE = {}


def _get_nc(reps=1, w_only=False):
    key = (reps, w_only)
    if key not in _NC_CACHE:
        _NC_CACHE[key] = _build_nc(reps, w_only)
    return _NC_CACHE[key]


def _prep_in_maps(x, weight, lora_A, lora_B):
    # x.T in SBUF partition-major layout: [4096,64] -> [KT,128,64] -> [128, KT*64]
    xt = np.ascontiguousarray(
        x.T.reshape(KT, P, TOK).transpose(1, 0, 2).reshape(P, KT * TOK))
    # rank-64 intermediate, identical on every core: uT[r, t]
    ut = np.ascontiguousarray((SCALING * (x @ lora_A.T)).T)
    # per-core blocked transposes: skips materializing the full 256 MiB
    # weight.T intermediate (~26% less host prep on this 1-CPU container)
    in_maps = []
    for c in range(N_CORES):
        sl = slice(c * O_SHARD, (c + 1) * O_SHARD)
        in_maps.append({
            "xt": xt,
            "ut": ut,
            "wt": np.ascontiguousarray(weight[sl, :].T),
            "bt": np.ascontiguousarray(lora_B[sl, :].T),
        })
    return in_maps


def kernel(x, weight, lora_A, lora_B, trace=False):
    x = np.asarray(x, dtype=np.float32)
    weight = np.asarray(weight, dtype=np.float32)
    lora_A = np.asarray(lora_A, dtype=np.float32)
    lora_B = np.asarray(lora_B, dtype=np.float32)
    nc = _get_nc()
    in_maps = _prep_in_maps(x, weight, lora_A, lora_B)
    res = run_bass_kernel_spmd(nc, in_maps, core_ids=list(range(N_CORES)),
                               trace=trace)
    out = np.concatenate([res.results[c]["out"] for c in range(N_CORES)], axis=1)
    if trace:
        kernel.last_results = res
    return out
